# revision 15
# baseline (speedup 1.0000x reference)
"""Trainium2 Bass kernel for a dense causal-attention transformer block.

Reference computation (fp32, B=2, S=2048, D=2048, H=16, HD=128):
    qkv = x @ Wqkv ; q,k,v split per head
    scores = (q @ k^T) * HD**-0.5, causal mask, softmax
    o = softmax(scores) @ v ; out = o @ Wo

Sharding: tensor-parallel over heads (4 groups of 4 heads) x data-parallel
over batch (2) = 8 cores. Each core computes a partial output projection
(its 512 o-channels x Wo rows); the host sums the 4 partials per batch.

Device layout tricks:
  - All matmul inputs are bf16 (4x faster PE than fp32); PSUM accum fp32.
  - qT/kT are produced channels-on-partitions so score tiles come out
    TRANSPOSED [keys=128, queries=512]; softmax sum is then a matmul with
    an all-ones lhsT (no cross-partition reduce, no transposes anywhere).
  - No max-subtraction in softmax: scores ~ N(0,1), exp is safe in fp32,
    and masked entries are multiplied by 0 after exp.
  - HD**-0.5 scaling folded into Wq on the host.
"""

import numpy as np
import ml_dtypes

BF16 = ml_dtypes.bfloat16

B = 2
S = 2048
D = 2048
H = 16
HD = 128
P = 128
G = 4            # TP groups (heads per group = 4)
NH = H // G      # heads per core = 4
CH = NH * HD     # o-channels per core = 512
NJ = S // 512    # 4 S-chunks of 512
KK = D // P      # 16 contraction tiles
ST = S // P      # 16 sequence row-tiles

_progs = {}

# normalizer strategy: "pe" = per-tile ones-matmuls on PE;
# "tree" = DVE+GpSimd pairwise tree; "tree_dve" = DVE-only tree;
# "pair" = one DVE/GpSimd pairwise level, then ones-matmuls on halved count
SUM_MODE = "tree"


def _build(repeat=1):
    """Build (once) the single-core Bass/Tile program shared by all 8 cores.

    repeat>1 executes the whole computation that many times inside one NEFF
    (used only for overhead-free timing via T(xN)-T(x1) differencing).
    """
    key = (repeat, SUM_MODE)
    if key in _progs:
        return _progs[key]

    import concourse.tile as tile
    from concourse import bacc, mybir

    f32 = mybir.dt.float32
    bf16 = mybir.dt.bfloat16
    EXP = mybir.ActivationFunctionType.Exp

    nc = bacc.Bacc("TRN2", target_bir_lowering=False, debug=False)

    # DRAM I/O, pre-packed on host so every DMA is contiguous per partition.
    # x:  [p, nj, kk, q]  = xT chunk layout (x[b].T tiled)
    # wq/wk: [p, mi, kk, m] (column-sharded Wqkv, q part prescaled by HD^-.5)
    # wv: [p, kk, n]      (rhs layout)
    # wo: [p, h, ncol, n] (row-sharded Wo)
    # masks: [k, j, q]    binary causal masks for the 4 diagonal positions
    # out: [p, si, col]   partial output (fp32)
    x_d = nc.dram_tensor("x", (P, NJ, KK, 512), bf16, kind="ExternalInput")
    wq_d = nc.dram_tensor("wq", (P, NH, KK, P), bf16, kind="ExternalInput")
    wk_d = nc.dram_tensor("wk", (P, NH, KK, P), bf16, kind="ExternalInput")
    wv_d = nc.dram_tensor("wv", (P, KK, CH), bf16, kind="ExternalInput")
    wo_d = nc.dram_tensor("wo", (P, NH, NJ, 512), bf16, kind="ExternalInput")
    mask_d = nc.dram_tensor("masks", (P, NH, 512), bf16, kind="ExternalInput")
    out_d = nc.dram_tensor("out", (P, ST, D), f32, kind="ExternalOutput")

    with tile.TileContext(nc) as tc:
        with (
            tc.tile_pool(name="persist", bufs=1) as pp,
            tc.tile_pool(name="psumA", bufs=4, space="PSUM") as psA,
            tc.tile_pool(name="psumB", bufs=2, space="PSUM") as psB,
        ):
            for rep in range(repeat):
                _emit_once(nc, tc, tile, mybir, pp, psA, psB,
                           x_d, wq_d, wk_d, wv_d, wo_d, mask_d, out_d,
                           f32, bf16, EXP, rep)

    nc.compile()
    _progs[key] = nc
    return nc


def _emit_once(nc, tc, tile, mybir, pp, psA, psB,
               x_d, wq_d, wk_d, wv_d, wo_d, mask_d, out_d,
               f32, bf16, EXP, rep):
    r = f"r{rep}_"
    # wq/wk as 4 per-head-group tiles so the first matmul group only
    # depends on a 0.5MB DMA, not the whole weight
    wq_t = [pp.tile([P, KK, P], bf16, name=f"{r}wq{mi}", tag=f"wq{mi}")
            for mi in range(NH)]
    wk_t = [pp.tile([P, KK, P], bf16, name=f"{r}wk{mi}", tag=f"wk{mi}")
            for mi in range(NH)]
    # wv (phase 1) and wo (phase 3) share one 16KB slot
    wv_sb = pp.tile([P, KK, CH], bf16, name=r + "wv_sb", tag="wvwo")
    q_sb = pp.tile([P, NH, S], bf16, name=r + "q_sb", tag="q")
    k_sb = pp.tile([P, NH, S], bf16, name=r + "k_sb", tag="k")
    v_sb = pp.tile([P, ST, CH], bf16, name=r + "v_sb", tag="v")
    o_sb = pp.tile([P, NH, S], bf16, name=r + "o_sb", tag="o")
    mask_sb = pp.tile([P, NH, 512], bf16, name=r + "mask_sb", tag="mask")
    ones_sb = pp.tile([P, P], bf16, name=r + "ones_sb", tag="ones")
    zbias = pp.tile([P, 1], f32, name=r + "zbias", tag="zbias")

    nc.gpsimd.memset(ones_sb[:], 1.0)
    nc.gpsimd.memset(zbias[:], 0.0)

    # ---------------- Phase 1: QKV projections ----------------
    with tc.tile_pool(name=r + "xpool", bufs=2) as xpool:
        xcs = {}
        # DMA issue order = arrival order: first x chunk (split in half)
        # and first weight slice land before everything else so PE can
        # start within a few us
        xcs[0] = xpool.tile([P, KK, 512], bf16, name=f"{r}xc0", tag="xc")
        nc.sync.dma_start(wq_t[0][:], wq_d[:, 0])
        for qtr in range(4):
            nc.sync.dma_start(
                xcs[0][:, qtr * KK // 4:(qtr + 1) * KK // 4],
                x_d[:, 0, qtr * KK // 4:(qtr + 1) * KK // 4])
        for mi in range(1, NH):
            nc.sync.dma_start(wq_t[mi][:], wq_d[:, mi])
        for mi in range(NH):
            nc.sync.dma_start(wk_t[mi][:], wk_d[:, mi])
        nc.sync.dma_start(wv_sb[:], wv_d[:])
        nc.sync.dma_start(mask_sb[:], mask_d[:])

        for nj in range(NJ):
            xc = xcs.get(nj)
            if xc is None:
                xc = xpool.tile([P, KK, 512], bf16, name=f"{r}xc{nj}",
                                tag="xc")
                nc.sync.dma_start(xc[:], x_d[:, nj])
            # qT, kT: [CH, S] channel-major (per head = 128 partitions)
            for w_t, dst in ((wq_t, q_sb), (wk_t, k_sb)):
                for mi in range(NH):
                    acc = psA.tile([P, 512], f32, name=f"{r}qk{nj}_{mi}",
                                   tag="accA")
                    for kk in range(KK):
                        nc.tensor.matmul(
                            acc[:], w_t[mi][:, kk, :], xc[:, kk, :],
                            start=(kk == 0), stop=(kk == KK - 1))
                    nc.scalar.copy(
                        out=dst[:, mi, nj * 512:(nj + 1) * 512],
                        in_=acc[:])
            # v: [S, CH] row-major (keys on partitions)
            for si in range(4):
                sg = 4 * nj + si
                acc = psA.tile([P, CH], f32, name=f"{r}v{sg}", tag="accA")
                for kk in range(KK):
                    nc.tensor.matmul(
                        acc[:], xc[:, kk, si * P:(si + 1) * P],
                        wv_sb[:, kk, :],
                        start=(kk == 0), stop=(kk == KK - 1))
                nc.scalar.copy(out=v_sb[:, sg, :], in_=acc[:])

    # wo reuses wv's slot (Tile serializes the DMA after last wv read)
    wo_sb = pp.tile([P, NH, NJ, 512], bf16, name=r + "wo_sb", tag="wvwo")
    nc.sync.dma_start(wo_sb[:], wo_d[:])

    # ---------- Phase 2+3: attention + output projection ----------
    ADD = mybir.AluOpType.add
    eng_toggle = [0]

    def tree_sum(tiles, tpool, key):
        """Pairwise-sum SBUF tiles on DVE/GpSimd (PE stays free); fp32
        intermediates, bf16 root (one rounding) so the final cross-partition
        ones-matmul runs at bf16 rate."""
        cur = list(tiles)
        lvl = 0
        while len(cur) > 1:
            nxt = []
            for i in range(0, len(cur) - 1, 2):
                root = len(cur) == 2
                t = tpool.tile([P, 512], bf16 if root else f32,
                               name=f"{key}_{lvl}_{i}",
                               tag="tsum_root" if root else "tsum")
                use_gps = (SUM_MODE == "tree" and eng_toggle[0] % 2 == 1)
                eng = nc.gpsimd if use_gps else nc.vector
                eng_toggle[0] += 1
                eng.tensor_tensor(t[:], cur[i][:], cur[i + 1][:], ADD)
                nxt.append(t)
            if len(cur) % 2:
                nxt.append(cur[-1])
            cur = nxt
            lvl += 1
        return cur[0]

    with (
        tc.tile_pool(name=r + "apool", bufs=20) as apool,
        tc.tile_pool(name=r + "tpool", bufs=10) as tpool,
        tc.tile_pool(name=r + "rpool", bufs=3) as rpool,
        tc.tile_pool(name=r + "ostage", bufs=4) as ostage,
    ):
        for qc in range(NJ):          # query chunk of 512
            qs, qe = qc * 512, (qc + 1) * 512
            ktmax = 4 * qc + 4        # causal: key tiles 0..ktmax-1
            for h in range(NH):
                # scoresT tiles [keys=128, queries=512] -> exp -> a
                a_tiles = []
                for kt in range(ktmax):
                    st = psA.tile([P, 512], f32,
                                  name=f"{r}st{qc}_{h}_{kt}", tag="accA")
                    nc.tensor.matmul(
                        st[:], k_sb[:, h, kt * P:(kt + 1) * P],
                        q_sb[:, h, qs:qe], start=True, stop=True)
                    a_t = apool.tile([P, 512], bf16,
                                     name=f"{r}a{qc}_{h}_{kt}", tag="a")
                    nc.scalar.activation(a_t[:], st[:], EXP,
                                         bias=zbias[:])
                    if kt >= 4 * qc:  # diagonal tile: apply causal 0/1
                        nc.vector.tensor_mul(
                            out=a_t[:], in0=a_t[:],
                            in1=mask_sb[:, kt - 4 * qc, :])
                    a_tiles.append(a_t)
                # oT accumulation: [HD, 512] += v_kt^T-contract a_kt
                po = psB.tile([P, 512], f32, name=f"{r}po{qc}_{h}",
                              tag="po")
                for kt in range(ktmax):
                    nc.tensor.matmul(
                        po[:], v_sb[:, kt, h * HD:(h + 1) * HD],
                        a_tiles[kt][:],
                        start=(kt == 0), stop=(kt == ktmax - 1))
                # normalizer: column sums of a over all key tiles, replicated
                # to all partitions by the all-ones lhsT
                pn = psB.tile([P, 512], f32, name=f"{r}pn{qc}_{h}",
                              tag="pn")
                if SUM_MODE == "pe":
                    for kt in range(ktmax):
                        nc.tensor.matmul(
                            pn[:], ones_sb[:], a_tiles[kt][:],
                            start=(kt == 0), stop=(kt == ktmax - 1))
                elif SUM_MODE == "pair":
                    pairs = []
                    for i in range(0, ktmax, 2):
                        t = tpool.tile([P, 512], bf16,
                                       name=f"{r}ts{qc}_{h}_{i}", tag="tsum")
                        eng = nc.vector if eng_toggle[0] % 2 == 0 else nc.gpsimd
                        eng_toggle[0] += 1
                        eng.tensor_tensor(t[:], a_tiles[i][:],
                                          a_tiles[i + 1][:], ADD)
                        pairs.append(t)
                    for i, t in enumerate(pairs):
                        nc.tensor.matmul(pn[:], ones_sb[:], t[:],
                                         start=(i == 0),
                                         stop=(i == len(pairs) - 1))
                else:
                    asum = tree_sum(a_tiles, tpool, f"{r}ts{qc}_{h}")
                    nc.tensor.matmul(pn[:], ones_sb[:], asum[:],
                                     start=True, stop=True)
                rec = rpool.tile([P, 512], f32, name=f"{r}rc{qc}_{h}",
                                 tag="rec")
                nc.vector.reciprocal_approx_fast(rec[:], pn[:])
                nc.vector.tensor_mul(out=o_sb[:, h, qs:qe],
                                     in0=po[:], in1=rec[:])

            # output projection, delayed one chunk so it never waits on the
            # o values produced just above
            for pqc in ([qc - 1] if qc > 0 else []) + ([qc] if qc == NJ - 1
                                                       else []):
                _emit_proj(nc, psA, ostage, o_sb, wo_sb, out_d, pqc, r)


def _emit_proj(nc, psA, ostage, o_sb, wo_sb, out_d, qc, r):
    import concourse.mybir as mybir
    f32 = mybir.dt.float32
    for si in range(4 * qc, 4 * qc + 4):
        for ncol in range(NJ):
            acc = psA.tile([P, 512], f32,
                           name=f"{r}pr{si}_{ncol}", tag="accA")
            for h in range(NH):
                nc.tensor.matmul(
                    acc[:], o_sb[:, h, si * P:(si + 1) * P],
                    wo_sb[:, h, ncol, :],
                    start=(h == 0), stop=(h == NH - 1))
            stg = ostage.tile([P, 512], f32,
                              name=f"{r}os{si}_{ncol}", tag="os")
            nc.vector.tensor_copy(out=stg[:], in_=acc[:])
            nc.sync.dma_start(
                out_d[:, si, ncol * 512:(ncol + 1) * 512],
                stg[:])


def _pack_inputs(x, Wqkv, Wo):
    """Host-side shard + pack into the per-core DMA-friendly layouts."""
    scale = np.float32(HD) ** np.float32(-0.5)
    masks = np.zeros((P, NH, 512), dtype=BF16)
    k_idx = np.arange(P)[:, None]
    q_idx = np.arange(512)[None, :]
    for j in range(NH):
        masks[:, j, :] = (P * j + k_idx <= q_idx).astype(BF16)

    in_maps = []
    for c in range(8):
        b, g = divmod(c, G)
        xb = np.asarray(x[b], dtype=np.float32)
        # xT packed: [p, nj, kk, q] with xT[128*kk+p, 512*nj+q] = xb[q', d']
        xp = np.ascontiguousarray(
            xb.astype(BF16).reshape(NJ, 512, KK, P).transpose(3, 0, 2, 1))
        wq = (np.asarray(Wqkv[:, CH * g:CH * (g + 1)], np.float32) * scale)
        wk = np.asarray(Wqkv[:, D + CH * g:D + CH * (g + 1)], np.float32)
        wv = np.asarray(Wqkv[:, 2 * D + CH * g:2 * D + CH * (g + 1)],
                        np.float32)
        wo = np.asarray(Wo[CH * g:CH * (g + 1), :], np.float32)
        wq_p = np.ascontiguousarray(
            wq.astype(BF16).reshape(KK, P, NH, P).transpose(1, 2, 0, 3))
        wk_p = np.ascontiguousarray(
            wk.astype(BF16).reshape(KK, P, NH, P).transpose(1, 2, 0, 3))
        wv_p = np.ascontiguousarray(
            wv.astype(BF16).reshape(KK, P, CH).transpose(1, 0, 2))
        wo_p = np.ascontiguousarray(
            wo.astype(BF16).reshape(NH, P, NJ, 512).transpose(1, 0, 2, 3))
        in_maps.append({
            "x": xp, "wq": wq_p, "wk": wk_p, "wv": wv_p, "wo": wo_p,
            "masks": masks,
        })
    return in_maps


def _unpack_outputs(results):
    """Sum the 4 TP partials per batch and restore [B, S, D]."""
    out = np.zeros((B, S, D), dtype=np.float32)
    for c, res in enumerate(results):
        b = c // G
        part = np.asarray(res["out"])           # [p, si, col]
        out[b] += part.transpose(1, 0, 2).reshape(S, D)
    return out


def kernel(x, Wqkv, Wo, _trace=False, _trace_kwargs=None):
    from concourse import bass_utils

    nc = _build()
    in_maps = _pack_inputs(x, Wqkv, Wo)
    res = bass_utils.run_bass_kernel_spmd(
        nc, in_maps, core_ids=list(range(8)), trace=_trace,
        **(_trace_kwargs or {}))
    out = _unpack_outputs(res.results)
    if _trace:
        kernel.last_result = res
    return out


# revision 16
# speedup vs baseline: 1.2128x; 1.2128x over previous
"""Trainium2 Bass kernel for a dense causal-attention transformer block.

Reference computation (fp32, B=2, S=2048, D=2048, H=16, HD=128):
    qkv = x @ Wqkv ; q,k,v split per head
    scores = (q @ k^T) * HD**-0.5, causal mask, softmax
    o = softmax(scores) @ v ; out = o @ Wo

Sharding: tensor-parallel over heads (4 groups of 4 heads) x data-parallel
over batch (2) = 8 cores. Each core computes a partial output projection
(its 512 o-channels x Wo rows); the host sums the 4 partials per batch.

Device layout tricks:
  - All matmul inputs are bf16 (4x faster PE than fp32); PSUM accum fp32.
  - qT/kT are produced channels-on-partitions so score tiles come out
    TRANSPOSED [keys=128, queries=512]; softmax sum is then a matmul with
    an all-ones lhsT (no cross-partition reduce, no transposes anywhere).
  - No max-subtraction in softmax: scores ~ N(0,1), exp is safe in fp32,
    and masked entries are multiplied by 0 after exp.
  - HD**-0.5 scaling folded into Wq on the host.
"""

import numpy as np
import ml_dtypes

BF16 = ml_dtypes.bfloat16

B = 2
S = 2048
D = 2048
H = 16
HD = 128
P = 128
G = 4            # TP groups (heads per group = 4)
NH = H // G      # heads per core = 4
CH = NH * HD     # o-channels per core = 512
NJ = S // 512    # 4 S-chunks of 512
KK = D // P      # 16 contraction tiles
ST = S // P      # 16 sequence row-tiles

_progs = {}

# normalizer strategy: "pe" = per-tile ones-matmuls on PE;
# "tree" = DVE+GpSimd pairwise tree; "tree_dve" = DVE-only tree;
# "pair" = one DVE/GpSimd pairwise level, then ones-matmuls on halved count
SUM_MODE = "tree"


def _build(repeat=1):
    """Build (once) the single-core Bass/Tile program shared by all 8 cores.

    repeat>1 executes the whole computation that many times inside one NEFF
    (used only for overhead-free timing via T(xN)-T(x1) differencing).
    """
    key = (repeat, SUM_MODE)
    if key in _progs:
        return _progs[key]

    import concourse.tile as tile
    from concourse import bacc, mybir

    f32 = mybir.dt.float32
    bf16 = mybir.dt.bfloat16
    EXP = mybir.ActivationFunctionType.Exp

    nc = bacc.Bacc("TRN2", target_bir_lowering=False, debug=False)

    # DRAM I/O, pre-packed on host so every DMA is contiguous per partition.
    # x:  [p, nj, kk, q]  = xT chunk layout (x[b].T tiled)
    # wq/wk: [p, mi, kk, m] (column-sharded Wqkv, q part prescaled by HD^-.5)
    # wv: [p, kk, n]      (rhs layout)
    # wo: [p, h, ncol, n] (row-sharded Wo)
    # masks: [k, j, q]    binary causal masks for the 4 diagonal positions
    # out: [p, si, col]   partial output (fp32)
    x_d = nc.dram_tensor("x", (P, NJ, KK, 512), bf16, kind="ExternalInput")
    wq_d = nc.dram_tensor("wq", (P, NH, KK, P), bf16, kind="ExternalInput")
    wk_d = nc.dram_tensor("wk", (P, NH, KK, P), bf16, kind="ExternalInput")
    wv_d = nc.dram_tensor("wv", (P, KK, CH), bf16, kind="ExternalInput")
    wo_d = nc.dram_tensor("wo", (P, NH, NJ, 512), bf16, kind="ExternalInput")
    mask_d = nc.dram_tensor("masks", (P, NH, 512), bf16, kind="ExternalInput")
    out_d = nc.dram_tensor("out", (P, ST, D), f32, kind="ExternalOutput")

    with tile.TileContext(nc) as tc:
        with (
            tc.tile_pool(name="persist", bufs=1) as pp,
            tc.tile_pool(name="psumA", bufs=4, space="PSUM") as psA,
            tc.tile_pool(name="psumB", bufs=2, space="PSUM") as psB,
        ):
            for rep in range(repeat):
                _emit_once(nc, tc, tile, mybir, pp, psA, psB,
                           x_d, wq_d, wk_d, wv_d, wo_d, mask_d, out_d,
                           f32, bf16, EXP, rep)

    nc.compile()
    _progs[key] = nc
    return nc


def _emit_once(nc, tc, tile, mybir, pp, psA, psB,
               x_d, wq_d, wk_d, wv_d, wo_d, mask_d, out_d,
               f32, bf16, EXP, rep):
    r = f"r{rep}_"
    # wq/wk as 4 per-head-group tiles so the first matmul group only
    # depends on a 0.5MB DMA, not the whole weight
    wq_t = [pp.tile([P, KK, P], bf16, name=f"{r}wq{mi}", tag=f"wq{mi}")
            for mi in range(NH)]
    wk_t = [pp.tile([P, KK, P], bf16, name=f"{r}wk{mi}", tag=f"wk{mi}")
            for mi in range(NH)]
    # wv (phase 1) and wo (phase 3) share one 16KB slot
    wv_sb = pp.tile([P, KK, CH], bf16, name=r + "wv_sb", tag="wvwo")
    q_sb = pp.tile([P, NH, S], bf16, name=r + "q_sb", tag="q")
    k_sb = pp.tile([P, NH, S], bf16, name=r + "k_sb", tag="k")
    v_sb = pp.tile([P, ST, CH], bf16, name=r + "v_sb", tag="v")
    o_sb = pp.tile([P, NH, S], bf16, name=r + "o_sb", tag="o")
    mask_sb = pp.tile([P, NH, 512], bf16, name=r + "mask_sb", tag="mask")
    ones_sb = pp.tile([P, P], bf16, name=r + "ones_sb", tag="ones")
    zbias = pp.tile([P, 1], f32, name=r + "zbias", tag="zbias")

    nc.gpsimd.memset(ones_sb[:], 1.0)
    nc.gpsimd.memset(zbias[:], 0.0)

    # ---------------- Phase 1: QKV projections ----------------
    with tc.tile_pool(name=r + "xpool", bufs=2) as xpool:
        xcs = {}
        # DMA issue order = arrival order: first x chunk (split in half)
        # and first weight slice land before everything else so PE can
        # start within a few us
        xcs[0] = xpool.tile([P, KK, 512], bf16, name=f"{r}xc0", tag="xc")
        nc.sync.dma_start(wq_t[0][:], wq_d[:, 0])
        for qtr in range(4):
            nc.sync.dma_start(
                xcs[0][:, qtr * KK // 4:(qtr + 1) * KK // 4],
                x_d[:, 0, qtr * KK // 4:(qtr + 1) * KK // 4])
        for mi in range(1, NH):
            nc.sync.dma_start(wq_t[mi][:], wq_d[:, mi])
        for mi in range(NH):
            nc.sync.dma_start(wk_t[mi][:], wk_d[:, mi])
        nc.sync.dma_start(wv_sb[:], wv_d[:])
        nc.sync.dma_start(mask_sb[:], mask_d[:])

        for nj in range(NJ):
            xc = xcs.get(nj)
            if xc is None:
                xc = xpool.tile([P, KK, 512], bf16, name=f"{r}xc{nj}",
                                tag="xc")
                nc.sync.dma_start(xc[:], x_d[:, nj])
            # qT, kT: [CH, S] channel-major (per head = 128 partitions)
            for w_t, dst in ((wq_t, q_sb), (wk_t, k_sb)):
                for mi in range(NH):
                    acc = psA.tile([P, 512], f32, name=f"{r}qk{nj}_{mi}",
                                   tag="accA")
                    for kk in range(KK):
                        nc.tensor.matmul(
                            acc[:], w_t[mi][:, kk, :], xc[:, kk, :],
                            start=(kk == 0), stop=(kk == KK - 1))
                    nc.scalar.copy(
                        out=dst[:, mi, nj * 512:(nj + 1) * 512],
                        in_=acc[:])
            # v: [S, CH] row-major (keys on partitions)
            for si in range(4):
                sg = 4 * nj + si
                acc = psA.tile([P, CH], f32, name=f"{r}v{sg}", tag="accA")
                for kk in range(KK):
                    nc.tensor.matmul(
                        acc[:], xc[:, kk, si * P:(si + 1) * P],
                        wv_sb[:, kk, :],
                        start=(kk == 0), stop=(kk == KK - 1))
                nc.scalar.copy(out=v_sb[:, sg, :], in_=acc[:])

    # wo reuses wv's slot (Tile serializes the DMA after last wv read)
    wo_sb = pp.tile([P, NH, NJ, 512], bf16, name=r + "wo_sb", tag="wvwo")
    nc.sync.dma_start(wo_sb[:], wo_d[:])

    # ---------- Phase 2+3: attention + output projection ----------
    ADD = mybir.AluOpType.add
    eng_toggle = [0]

    def tree_sum(tiles, tpool, key):
        """Pairwise-sum SBUF tiles on DVE/GpSimd (PE stays free); fp32
        intermediates, bf16 root (one rounding) so the final cross-partition
        ones-matmul runs at bf16 rate."""
        cur = list(tiles)
        lvl = 0
        while len(cur) > 1:
            nxt = []
            for i in range(0, len(cur) - 1, 2):
                root = len(cur) == 2
                t = tpool.tile([P, 512], bf16 if root else f32,
                               name=f"{key}_{lvl}_{i}",
                               tag="tsum_root" if root else "tsum")
                use_gps = (SUM_MODE == "tree" and eng_toggle[0] % 2 == 1)
                eng = nc.gpsimd if use_gps else nc.vector
                eng_toggle[0] += 1
                eng.tensor_tensor(t[:], cur[i][:], cur[i + 1][:], ADD)
                nxt.append(t)
            if len(cur) % 2:
                nxt.append(cur[-1])
            cur = nxt
            lvl += 1
        return cur[0]

    with (
        tc.tile_pool(name=r + "apool", bufs=20) as apool,
        tc.tile_pool(name=r + "tpool", bufs=10) as tpool,
        tc.tile_pool(name=r + "rpool", bufs=3) as rpool,
        tc.tile_pool(name=r + "ostage", bufs=4) as ostage,
    ):
        for qc in range(NJ):          # query chunk of 512
            qs, qe = qc * 512, (qc + 1) * 512
            ktmax = 4 * qc + 4        # causal: key tiles 0..ktmax-1
            for h in range(NH):
                # scoresT tiles [keys=128, queries=512] -> exp -> a
                a_tiles = []
                for kt in range(ktmax):
                    st = psA.tile([P, 512], f32,
                                  name=f"{r}st{qc}_{h}_{kt}", tag="accA")
                    nc.tensor.matmul(
                        st[:], k_sb[:, h, kt * P:(kt + 1) * P],
                        q_sb[:, h, qs:qe], start=True, stop=True)
                    a_t = apool.tile([P, 512], bf16,
                                     name=f"{r}a{qc}_{h}_{kt}", tag="a")
                    nc.scalar.activation(a_t[:], st[:], EXP,
                                         bias=zbias[:])
                    if kt >= 4 * qc:  # diagonal tile: apply causal 0/1
                        nc.vector.tensor_mul(
                            out=a_t[:], in0=a_t[:],
                            in1=mask_sb[:, kt - 4 * qc, :])
                    a_tiles.append(a_t)
                # oT accumulation: [HD, 512] += v_kt^T-contract a_kt
                po = psB.tile([P, 512], f32, name=f"{r}po{qc}_{h}",
                              tag="po")
                for kt in range(ktmax):
                    nc.tensor.matmul(
                        po[:], v_sb[:, kt, h * HD:(h + 1) * HD],
                        a_tiles[kt][:],
                        start=(kt == 0), stop=(kt == ktmax - 1))
                # normalizer: column sums of a over all key tiles, replicated
                # to all partitions by the all-ones lhsT
                pn = psB.tile([P, 512], f32, name=f"{r}pn{qc}_{h}",
                              tag="pn")
                if SUM_MODE == "pe":
                    for kt in range(ktmax):
                        nc.tensor.matmul(
                            pn[:], ones_sb[:], a_tiles[kt][:],
                            start=(kt == 0), stop=(kt == ktmax - 1))
                elif SUM_MODE == "pair":
                    pairs = []
                    for i in range(0, ktmax, 2):
                        t = tpool.tile([P, 512], bf16,
                                       name=f"{r}ts{qc}_{h}_{i}", tag="tsum")
                        eng = nc.vector if eng_toggle[0] % 2 == 0 else nc.gpsimd
                        eng_toggle[0] += 1
                        eng.tensor_tensor(t[:], a_tiles[i][:],
                                          a_tiles[i + 1][:], ADD)
                        pairs.append(t)
                    for i, t in enumerate(pairs):
                        nc.tensor.matmul(pn[:], ones_sb[:], t[:],
                                         start=(i == 0),
                                         stop=(i == len(pairs) - 1))
                else:
                    asum = tree_sum(a_tiles, tpool, f"{r}ts{qc}_{h}")
                    nc.tensor.matmul(pn[:], ones_sb[:], asum[:],
                                     start=True, stop=True)
                rec = rpool.tile([P, 512], f32, name=f"{r}rc{qc}_{h}",
                                 tag="rec")
                nc.vector.reciprocal_approx_fast(rec[:], pn[:])
                nc.vector.tensor_mul(out=o_sb[:, h, qs:qe],
                                     in0=po[:], in1=rec[:])

            # output projection, delayed one chunk so it never waits on the
            # o values produced just above
            for pqc in ([qc - 1] if qc > 0 else []) + ([qc] if qc == NJ - 1
                                                       else []):
                _emit_proj(nc, psA, ostage, o_sb, wo_sb, out_d, pqc, r)


def _emit_proj(nc, psA, ostage, o_sb, wo_sb, out_d, qc, r):
    import concourse.mybir as mybir
    f32 = mybir.dt.float32
    for si in range(4 * qc, 4 * qc + 4):
        for ncol in range(NJ):
            acc = psA.tile([P, 512], f32,
                           name=f"{r}pr{si}_{ncol}", tag="accA")
            for h in range(NH):
                nc.tensor.matmul(
                    acc[:], o_sb[:, h, si * P:(si + 1) * P],
                    wo_sb[:, h, ncol, :],
                    start=(h == 0), stop=(h == NH - 1))
            stg = ostage.tile([P, 512], f32,
                              name=f"{r}os{si}_{ncol}", tag="os")
            nc.vector.tensor_copy(out=stg[:], in_=acc[:])
            nc.sync.dma_start(
                out_d[:, si, ncol * 512:(ncol + 1) * 512],
                stg[:])


def _pack_inputs(x, Wqkv, Wo):
    """Host-side shard + pack into the per-core DMA-friendly layouts.
    Arrays are shared between cores where identical (x per batch, weights
    per TP group, masks global)."""
    scale = np.float32(HD) ** np.float32(-0.5)
    masks = np.zeros((P, NH, 512), dtype=BF16)
    k_idx = np.arange(P)[:, None]
    q_idx = np.arange(512)[None, :]
    for j in range(NH):
        masks[:, j, :] = (P * j + k_idx <= q_idx).astype(BF16)

    xps = []
    for b in range(B):
        xb = np.asarray(x[b], dtype=np.float32)
        # xT packed: [p, nj, kk, q] with xT[128*kk+p, 512*nj+q] = xb[q', d']
        xps.append(np.ascontiguousarray(
            xb.astype(BF16).reshape(NJ, 512, KK, P).transpose(3, 0, 2, 1)))

    wmaps = []
    for g in range(G):
        wq = (np.asarray(Wqkv[:, CH * g:CH * (g + 1)], np.float32) * scale)
        wk = np.asarray(Wqkv[:, D + CH * g:D + CH * (g + 1)], np.float32)
        wv = np.asarray(Wqkv[:, 2 * D + CH * g:2 * D + CH * (g + 1)],
                        np.float32)
        wo = np.asarray(Wo[CH * g:CH * (g + 1), :], np.float32)
        wmaps.append({
            "wq": np.ascontiguousarray(
                wq.astype(BF16).reshape(KK, P, NH, P).transpose(1, 2, 0, 3)),
            "wk": np.ascontiguousarray(
                wk.astype(BF16).reshape(KK, P, NH, P).transpose(1, 2, 0, 3)),
            "wv": np.ascontiguousarray(
                wv.astype(BF16).reshape(KK, P, CH).transpose(1, 0, 2)),
            "wo": np.ascontiguousarray(
                wo.astype(BF16).reshape(NH, P, NJ, 512).transpose(1, 0, 2, 3)),
        })

    return [{"x": xps[c // G], "masks": masks, **wmaps[c % G]}
            for c in range(8)]


def _unpack_outputs(results):
    """Sum the 4 TP partials per batch and restore [B, S, D]."""
    out = np.zeros((B, S, D), dtype=np.float32)
    for c, res in enumerate(results):
        b = c // G
        part = np.asarray(res["out"])           # [p, si, col]
        out[b] += part.transpose(1, 0, 2).reshape(S, D)
    return out


def kernel(x, Wqkv, Wo, _trace=False, _trace_kwargs=None):
    from concourse import bass_utils

    nc = _build()
    in_maps = _pack_inputs(x, Wqkv, Wo)
    res = bass_utils.run_bass_kernel_spmd(
        nc, in_maps, core_ids=list(range(8)), trace=_trace,
        **(_trace_kwargs or {}))
    out = _unpack_outputs(res.results)
    if _trace:
        kernel.last_result = res
    return out


# revision 19
# speedup vs baseline: 1.2214x; 1.0071x over previous
"""Trainium2 Bass kernel for a dense causal-attention transformer block.

Reference computation (fp32, B=2, S=2048, D=2048, H=16, HD=128):
    qkv = x @ Wqkv ; q,k,v split per head
    scores = (q @ k^T) * HD**-0.5, causal mask, softmax
    o = softmax(scores) @ v ; out = o @ Wo

Sharding: tensor-parallel over heads (4 groups of 4 heads) x data-parallel
over batch (2) = 8 cores. Each core computes a partial output projection
(its 512 o-channels x Wo rows); the host sums the 4 partials per batch.

Device layout tricks:
  - All matmul inputs are bf16 (4x faster PE than fp32); PSUM accum fp32.
  - qT/kT are produced channels-on-partitions so score tiles come out
    TRANSPOSED [keys=128, queries=512]; softmax sum is then a matmul with
    an all-ones lhsT (no cross-partition reduce, no transposes anywhere).
  - No max-subtraction in softmax: scores ~ N(0,1), exp is safe in fp32,
    and masked entries are multiplied by 0 after exp.
  - HD**-0.5 scaling folded into Wq on the host.
"""

import numpy as np
import ml_dtypes

BF16 = ml_dtypes.bfloat16

B = 2
S = 2048
D = 2048
H = 16
HD = 128
P = 128
G = 4            # TP groups (heads per group = 4)
NH = H // G      # heads per core = 4
CH = NH * HD     # o-channels per core = 512
NJ = S // 512    # 4 S-chunks of 512
KK = D // P      # 16 contraction tiles
ST = S // P      # 16 sequence row-tiles

_progs = {}

# normalizer strategy: "pe" = per-tile ones-matmuls on PE;
# "tree" = DVE+GpSimd pairwise tree; "tree_dve" = DVE-only tree;
# "pair" = one DVE/GpSimd pairwise level, then ones-matmuls on halved count
SUM_MODE = "tree"


def _build(repeat=1):
    """Build (once) the single-core Bass/Tile program shared by all 8 cores.

    repeat>1 executes the whole computation that many times inside one NEFF
    (used only for overhead-free timing via T(xN)-T(x1) differencing).
    """
    key = (repeat, SUM_MODE)
    if key in _progs:
        return _progs[key]

    import concourse.tile as tile
    from concourse import bacc, mybir

    f32 = mybir.dt.float32
    bf16 = mybir.dt.bfloat16
    EXP = mybir.ActivationFunctionType.Exp

    nc = bacc.Bacc("TRN2", target_bir_lowering=False, debug=False)

    # DRAM I/O, pre-packed on host so every DMA is contiguous per partition.
    # x:  [p, nj, kk, q]  = xT chunk layout (x[b].T tiled)
    # wq/wk: [p, mi, kk, m] (column-sharded Wqkv, q part prescaled by HD^-.5)
    # wv: [p, kk, n]      (rhs layout)
    # wo: [p, h, ncol, n] (row-sharded Wo)
    # masks: [k, j, q]    binary causal masks for the 4 diagonal positions
    # out: [p, si, col]   partial output (fp32)
    x_d = nc.dram_tensor("x", (P, NJ, KK, 512), bf16, kind="ExternalInput")
    wq_d = nc.dram_tensor("wq", (P, NH, KK, P), bf16, kind="ExternalInput")
    wk_d = nc.dram_tensor("wk", (P, NH, KK, P), bf16, kind="ExternalInput")
    wv_d = nc.dram_tensor("wv", (P, KK, CH), bf16, kind="ExternalInput")
    wo_d = nc.dram_tensor("wo", (P, NH, NJ, 512), bf16, kind="ExternalInput")
    mask_d = nc.dram_tensor("masks", (P, NH, 512), bf16, kind="ExternalInput")
    # partial outputs in bf16 (halves output DMA); host sums them in fp32
    out_d = nc.dram_tensor("out", (P, ST, D), bf16, kind="ExternalOutput")

    with tile.TileContext(nc) as tc:
        with (
            tc.tile_pool(name="persist", bufs=1) as pp,
            tc.tile_pool(name="psumA", bufs=4, space="PSUM") as psA,
            tc.tile_pool(name="psumB", bufs=2, space="PSUM") as psB,
        ):
            for rep in range(repeat):
                _emit_once(nc, tc, tile, mybir, pp, psA, psB,
                           x_d, wq_d, wk_d, wv_d, wo_d, mask_d, out_d,
                           f32, bf16, EXP, rep)

    nc.compile()
    _progs[key] = nc
    return nc


def _emit_once(nc, tc, tile, mybir, pp, psA, psB,
               x_d, wq_d, wk_d, wv_d, wo_d, mask_d, out_d,
               f32, bf16, EXP, rep):
    r = f"r{rep}_"
    # wq/wk as 4 per-head-group tiles so the first matmul group only
    # depends on a 0.5MB DMA, not the whole weight
    wq_t = [pp.tile([P, KK, P], bf16, name=f"{r}wq{mi}", tag=f"wq{mi}")
            for mi in range(NH)]
    wk_t = [pp.tile([P, KK, P], bf16, name=f"{r}wk{mi}", tag=f"wk{mi}")
            for mi in range(NH)]
    # wv (phase 1) and wo (phase 3) share one 16KB slot
    wv_sb = pp.tile([P, KK, CH], bf16, name=r + "wv_sb", tag="wvwo")
    q_sb = pp.tile([P, NH, S], bf16, name=r + "q_sb", tag="q")
    k_sb = pp.tile([P, NH, S], bf16, name=r + "k_sb", tag="k")
    v_sb = pp.tile([P, ST, CH], bf16, name=r + "v_sb", tag="v")
    o_sb = pp.tile([P, NH, S], bf16, name=r + "o_sb", tag="o")
    mask_sb = pp.tile([P, NH, 512], bf16, name=r + "mask_sb", tag="mask")
    ones_sb = pp.tile([P, P], bf16, name=r + "ones_sb", tag="ones")
    zbias = pp.tile([P, 1], f32, name=r + "zbias", tag="zbias")

    nc.gpsimd.memset(ones_sb[:], 1.0)
    nc.gpsimd.memset(zbias[:], 0.0)

    # ---------------- Phase 1: QKV projections ----------------
    with tc.tile_pool(name=r + "xpool", bufs=2) as xpool:
        xcs = {}
        # DMA issue order = arrival order: first x chunk (split in half)
        # and first weight slice land before everything else so PE can
        # start within a few us
        xcs[0] = xpool.tile([P, KK, 512], bf16, name=f"{r}xc0", tag="xc")
        nc.sync.dma_start(wq_t[0][:], wq_d[:, 0])
        for qtr in range(4):
            nc.sync.dma_start(
                xcs[0][:, qtr * KK // 4:(qtr + 1) * KK // 4],
                x_d[:, 0, qtr * KK // 4:(qtr + 1) * KK // 4])
        for mi in range(1, NH):
            nc.sync.dma_start(wq_t[mi][:], wq_d[:, mi])
        for mi in range(NH):
            nc.sync.dma_start(wk_t[mi][:], wk_d[:, mi])
        nc.sync.dma_start(wv_sb[:], wv_d[:])
        nc.sync.dma_start(mask_sb[:], mask_d[:])

        for nj in range(NJ):
            xc = xcs.get(nj)
            if xc is None:
                xc = xpool.tile([P, KK, 512], bf16, name=f"{r}xc{nj}",
                                tag="xc")
                nc.sync.dma_start(xc[:], x_d[:, nj])
            # qT, kT: [CH, S] channel-major (per head = 128 partitions)
            for w_t, dst in ((wq_t, q_sb), (wk_t, k_sb)):
                for mi in range(NH):
                    acc = psA.tile([P, 512], f32, name=f"{r}qk{nj}_{mi}",
                                   tag="accA")
                    for kk in range(KK):
                        nc.tensor.matmul(
                            acc[:], w_t[mi][:, kk, :], xc[:, kk, :],
                            start=(kk == 0), stop=(kk == KK - 1))
                    nc.scalar.copy(
                        out=dst[:, mi, nj * 512:(nj + 1) * 512],
                        in_=acc[:])
            # v: [S, CH] row-major (keys on partitions)
            for si in range(4):
                sg = 4 * nj + si
                acc = psA.tile([P, CH], f32, name=f"{r}v{sg}", tag="accA")
                for kk in range(KK):
                    nc.tensor.matmul(
                        acc[:], xc[:, kk, si * P:(si + 1) * P],
                        wv_sb[:, kk, :],
                        start=(kk == 0), stop=(kk == KK - 1))
                nc.scalar.copy(out=v_sb[:, sg, :], in_=acc[:])

    # wo reuses wv's slot (Tile serializes the DMA after last wv read)
    wo_sb = pp.tile([P, NH, NJ, 512], bf16, name=r + "wo_sb", tag="wvwo")
    nc.sync.dma_start(wo_sb[:], wo_d[:])

    # ---------- Phase 2+3: attention + output projection ----------
    ADD = mybir.AluOpType.add
    eng_toggle = [0]

    def tree_sum(tiles, tpool, key):
        """Pairwise-sum SBUF tiles on DVE/GpSimd (PE stays free); fp32
        intermediates, bf16 root (one rounding) so the final cross-partition
        ones-matmul runs at bf16 rate."""
        cur = list(tiles)
        lvl = 0
        while len(cur) > 1:
            nxt = []
            for i in range(0, len(cur) - 1, 2):
                root = len(cur) == 2
                t = tpool.tile([P, 512], bf16 if root else f32,
                               name=f"{key}_{lvl}_{i}",
                               tag="tsum_root" if root else "tsum")
                use_gps = (SUM_MODE == "tree" and eng_toggle[0] % 2 == 1)
                eng = nc.gpsimd if use_gps else nc.vector
                eng_toggle[0] += 1
                eng.tensor_tensor(t[:], cur[i][:], cur[i + 1][:], ADD)
                nxt.append(t)
            if len(cur) % 2:
                nxt.append(cur[-1])
            cur = nxt
            lvl += 1
        return cur[0]

    with (
        tc.tile_pool(name=r + "apool", bufs=20) as apool,
        tc.tile_pool(name=r + "tpool", bufs=10) as tpool,
        tc.tile_pool(name=r + "rpool", bufs=3) as rpool,
        tc.tile_pool(name=r + "ostage", bufs=4) as ostage,
    ):
        for qc in range(NJ):          # query chunk of 512
            qs, qe = qc * 512, (qc + 1) * 512
            ktmax = 4 * qc + 4        # causal: key tiles 0..ktmax-1
            for h in range(NH):
                # scoresT tiles [keys=128, queries=512] -> exp -> a
                a_tiles = []
                for kt in range(ktmax):
                    st = psA.tile([P, 512], f32,
                                  name=f"{r}st{qc}_{h}_{kt}", tag="accA")
                    nc.tensor.matmul(
                        st[:], k_sb[:, h, kt * P:(kt + 1) * P],
                        q_sb[:, h, qs:qe], start=True, stop=True)
                    a_t = apool.tile([P, 512], bf16,
                                     name=f"{r}a{qc}_{h}_{kt}", tag="a")
                    nc.scalar.activation(a_t[:], st[:], EXP,
                                         bias=zbias[:])
                    if kt >= 4 * qc:  # diagonal tile: apply causal 0/1
                        nc.vector.tensor_mul(
                            out=a_t[:], in0=a_t[:],
                            in1=mask_sb[:, kt - 4 * qc, :])
                    a_tiles.append(a_t)
                # oT accumulation: [HD, 512] += v_kt^T-contract a_kt
                po = psB.tile([P, 512], f32, name=f"{r}po{qc}_{h}",
                              tag="po")
                for kt in range(ktmax):
                    nc.tensor.matmul(
                        po[:], v_sb[:, kt, h * HD:(h + 1) * HD],
                        a_tiles[kt][:],
                        start=(kt == 0), stop=(kt == ktmax - 1))
                # normalizer: column sums of a over all key tiles, replicated
                # to all partitions by the all-ones lhsT
                pn = psB.tile([P, 512], f32, name=f"{r}pn{qc}_{h}",
                              tag="pn")
                if SUM_MODE == "pe":
                    for kt in range(ktmax):
                        nc.tensor.matmul(
                            pn[:], ones_sb[:], a_tiles[kt][:],
                            start=(kt == 0), stop=(kt == ktmax - 1))
                elif SUM_MODE == "pair":
                    pairs = []
                    for i in range(0, ktmax, 2):
                        t = tpool.tile([P, 512], bf16,
                                       name=f"{r}ts{qc}_{h}_{i}", tag="tsum")
                        eng = nc.vector if eng_toggle[0] % 2 == 0 else nc.gpsimd
                        eng_toggle[0] += 1
                        eng.tensor_tensor(t[:], a_tiles[i][:],
                                          a_tiles[i + 1][:], ADD)
                        pairs.append(t)
                    for i, t in enumerate(pairs):
                        nc.tensor.matmul(pn[:], ones_sb[:], t[:],
                                         start=(i == 0),
                                         stop=(i == len(pairs) - 1))
                else:
                    asum = tree_sum(a_tiles, tpool, f"{r}ts{qc}_{h}")
                    nc.tensor.matmul(pn[:], ones_sb[:], asum[:],
                                     start=True, stop=True)
                rec = rpool.tile([P, 512], f32, name=f"{r}rc{qc}_{h}",
                                 tag="rec")
                nc.vector.reciprocal_approx_fast(rec[:], pn[:])
                nc.vector.tensor_mul(out=o_sb[:, h, qs:qe],
                                     in0=po[:], in1=rec[:])

            # output projection, delayed one chunk so it never waits on the
            # o values produced just above
            for pqc in ([qc - 1] if qc > 0 else []) + ([qc] if qc == NJ - 1
                                                       else []):
                _emit_proj(nc, psA, ostage, o_sb, wo_sb, out_d, pqc, r)


def _emit_proj(nc, psA, ostage, o_sb, wo_sb, out_d, qc, r):
    import concourse.mybir as mybir
    f32 = mybir.dt.float32
    bf16 = mybir.dt.bfloat16
    for si in range(4 * qc, 4 * qc + 4):
        for ncol in range(NJ):
            acc = psA.tile([P, 512], f32,
                           name=f"{r}pr{si}_{ncol}", tag="accA")
            for h in range(NH):
                nc.tensor.matmul(
                    acc[:], o_sb[:, h, si * P:(si + 1) * P],
                    wo_sb[:, h, ncol, :],
                    start=(h == 0), stop=(h == NH - 1))
            stg = ostage.tile([P, 512], bf16,
                              name=f"{r}os{si}_{ncol}", tag="os")
            nc.vector.tensor_copy(out=stg[:], in_=acc[:])
            nc.sync.dma_start(
                out_d[:, si, ncol * 512:(ncol + 1) * 512],
                stg[:])


def _pack_inputs(x, Wqkv, Wo):
    """Host-side shard + pack into the per-core DMA-friendly layouts.
    Arrays are shared between cores where identical (x per batch, weights
    per TP group, masks global)."""
    scale = np.float32(HD) ** np.float32(-0.5)
    masks = np.zeros((P, NH, 512), dtype=BF16)
    k_idx = np.arange(P)[:, None]
    q_idx = np.arange(512)[None, :]
    for j in range(NH):
        masks[:, j, :] = (P * j + k_idx <= q_idx).astype(BF16)

    xps = []
    for b in range(B):
        xb = np.asarray(x[b], dtype=np.float32)
        # xT packed: [p, nj, kk, q] with xT[128*kk+p, 512*nj+q] = xb[q', d']
        xps.append(np.ascontiguousarray(
            xb.astype(BF16).reshape(NJ, 512, KK, P).transpose(3, 0, 2, 1)))

    wmaps = []
    for g in range(G):
        wq = (np.asarray(Wqkv[:, CH * g:CH * (g + 1)], np.float32) * scale)
        wk = np.asarray(Wqkv[:, D + CH * g:D + CH * (g + 1)], np.float32)
        wv = np.asarray(Wqkv[:, 2 * D + CH * g:2 * D + CH * (g + 1)],
                        np.float32)
        wo = np.asarray(Wo[CH * g:CH * (g + 1), :], np.float32)
        wmaps.append({
            "wq": np.ascontiguousarray(
                wq.astype(BF16).reshape(KK, P, NH, P).transpose(1, 2, 0, 3)),
            "wk": np.ascontiguousarray(
                wk.astype(BF16).reshape(KK, P, NH, P).transpose(1, 2, 0, 3)),
            "wv": np.ascontiguousarray(
                wv.astype(BF16).reshape(KK, P, CH).transpose(1, 0, 2)),
            "wo": np.ascontiguousarray(
                wo.astype(BF16).reshape(NH, P, NJ, 512).transpose(1, 0, 2, 3)),
        })

    return [{"x": xps[c // G], "masks": masks, **wmaps[c % G]}
            for c in range(8)]


def _unpack_outputs(results):
    """Sum the 4 TP partials per batch and restore [B, S, D]."""
    out = np.zeros((B, S, D), dtype=np.float32)
    for c, res in enumerate(results):
        b = c // G
        part = np.asarray(res["out"]).astype(np.float32)   # [p, si, col]
        out[b] += part.transpose(1, 0, 2).reshape(S, D)
    return out


def kernel(x, Wqkv, Wo, _trace=False, _trace_kwargs=None):
    from concourse import bass_utils

    nc = _build()
    in_maps = _pack_inputs(x, Wqkv, Wo)
    res = bass_utils.run_bass_kernel_spmd(
        nc, in_maps, core_ids=list(range(8)), trace=_trace,
        **(_trace_kwargs or {}))
    out = _unpack_outputs(res.results)
    if _trace:
        kernel.last_result = res
    return out


# revision 23
# speedup vs baseline: 1.5477x; 1.2672x over previous
"""Trainium2 Bass kernel for a dense causal-attention transformer block.

Reference computation (fp32, B=2, S=2048, D=2048, H=16, HD=128):
    qkv = x @ Wqkv ; q,k,v split per head
    scores = (q @ k^T) * HD**-0.5, causal mask, softmax
    o = softmax(scores) @ v ; out = o @ Wo

Sharding: tensor-parallel over heads (4 groups of 4 heads) x data-parallel
over batch (2) = 8 cores. Each core computes a partial output projection
(its 512 o-channels x Wo rows); the host sums the 4 partials per batch.

Device layout tricks:
  - All matmul inputs are bf16 (4x faster PE than fp32); PSUM accum fp32.
  - qT/kT are produced channels-on-partitions so score tiles come out
    TRANSPOSED [keys=128, queries=512]; softmax sum is then a matmul with
    an all-ones lhsT (no cross-partition reduce, no transposes anywhere).
  - No max-subtraction in softmax: scores ~ N(0,1), exp is safe in fp32,
    and masked entries are multiplied by 0 after exp.
  - HD**-0.5 scaling folded into Wq on the host.
"""

import numpy as np
import ml_dtypes

BF16 = ml_dtypes.bfloat16

B = 2
S = 2048
D = 2048
H = 16
HD = 128
P = 128
G = 4            # TP groups (heads per group = 4)
NH = H // G      # heads per core = 4
CH = NH * HD     # o-channels per core = 512
NJ = S // 512    # 4 S-chunks of 512
KK = D // P      # 16 contraction tiles
ST = S // P      # 16 sequence row-tiles

_progs = {}

# normalizer strategy: "pe" = per-tile ones-matmuls on PE;
# "pair" = one DVE/GpSimd pairwise-add level, then half as many ones-matmuls
SUM_MODE = "pair"


def _build(repeat=1):
    """Build (once) the single-core Bass/Tile program shared by all 8 cores.

    repeat>1 executes the whole computation that many times inside one NEFF
    (used only for overhead-free timing via T(xN)-T(x1) differencing).
    """
    key = (repeat, SUM_MODE)
    if key in _progs:
        return _progs[key]

    import concourse.tile as tile
    from concourse import bacc, mybir

    f32 = mybir.dt.float32
    bf16 = mybir.dt.bfloat16
    EXP = mybir.ActivationFunctionType.Exp

    nc = bacc.Bacc("TRN2", target_bir_lowering=False, debug=False)

    # DRAM I/O, pre-packed on host so every DMA is contiguous per partition.
    # x:  [p, nj, kk, q]  = xT chunk layout (x[b].T tiled)
    # wq/wk: [p, mi, kk, m] (column-sharded Wqkv, q part prescaled by HD^-.5)
    # wv: [p, kk, n]      (rhs layout)
    # wo: [p, h, ncol, n] (row-sharded Wo)
    # masks: [k, j, q]    binary causal masks for the 4 diagonal positions
    # out: [p, si, col]   partial output (fp32)
    x_d = nc.dram_tensor("x", (P, NJ, KK, 512), bf16, kind="ExternalInput")
    wq_d = nc.dram_tensor("wq", (P, NH, KK, P), bf16, kind="ExternalInput")
    wk_d = nc.dram_tensor("wk", (P, NH, KK, P), bf16, kind="ExternalInput")
    wv_d = nc.dram_tensor("wv", (P, KK, CH), bf16, kind="ExternalInput")
    wo_d = nc.dram_tensor("wo", (P, NH, NJ, 512), bf16, kind="ExternalInput")
    mask_d = nc.dram_tensor("masks", (P, NH, 512), bf16, kind="ExternalInput")
    # partial outputs in bf16 (halves output DMA); host sums them in fp32
    out_d = nc.dram_tensor("out", (P, ST, D), bf16, kind="ExternalOutput")

    with tile.TileContext(nc) as tc:
        with (
            tc.tile_pool(name="persist", bufs=1) as pp,
            tc.tile_pool(name="psumA", bufs=2, space="PSUM") as psA,
            tc.tile_pool(name="psumW", bufs=2, space="PSUM") as psW,
            tc.tile_pool(name="psumB", bufs=1, space="PSUM") as psB,
        ):
            for rep in range(repeat):
                _emit_once(nc, tc, tile, mybir, pp, psA, psW, psB,
                           x_d, wq_d, wk_d, wv_d, wo_d, mask_d, out_d,
                           f32, bf16, EXP, rep)

    nc.compile()
    _progs[key] = nc
    return nc


def _emit_once(nc, tc, tile, mybir, pp, psA, psW, psB,
               x_d, wq_d, wk_d, wv_d, wo_d, mask_d, out_d,
               f32, bf16, EXP, rep):
    r = f"r{rep}_"
    # wq/wk as 4 per-head-group tiles so the first matmul group only
    # depends on a 0.5MB DMA, not the whole weight
    wq_t = [pp.tile([P, KK, P], bf16, name=f"{r}wq{mi}", tag=f"wq{mi}")
            for mi in range(NH)]
    wk_t = [pp.tile([P, KK, P], bf16, name=f"{r}wk{mi}", tag=f"wk{mi}")
            for mi in range(NH)]
    # wv (phase 1) and wo (phase 3) share one 16KB slot
    wv_sb = pp.tile([P, KK, CH], bf16, name=r + "wv_sb", tag="wvwo")
    q_sb = pp.tile([P, NH, S], bf16, name=r + "q_sb", tag="q")
    k_sb = pp.tile([P, NH, S], bf16, name=r + "k_sb", tag="k")
    v_sb = pp.tile([P, ST, CH], bf16, name=r + "v_sb", tag="v")
    o_sb = pp.tile([P, NH, S], bf16, name=r + "o_sb", tag="o")
    mask_sb = pp.tile([P, NH, 512], bf16, name=r + "mask_sb", tag="mask")
    ones_sb = pp.tile([P, P], bf16, name=r + "ones_sb", tag="ones")
    zbias = pp.tile([P, 1], f32, name=r + "zbias", tag="zbias")

    nc.gpsimd.memset(ones_sb[:], 1.0)
    nc.gpsimd.memset(zbias[:], 0.0)

    # ---------------- Phase 1: QKV projections ----------------
    with tc.tile_pool(name=r + "xpool", bufs=2) as xpool:
        xcs = {}
        # DMA issue order = arrival order: first x chunk (split in half)
        # and first weight slice land before everything else so PE can
        # start within a few us
        xcs[0] = xpool.tile([P, KK, 512], bf16, name=f"{r}xc0", tag="xc")
        nc.sync.dma_start(wq_t[0][:], wq_d[:, 0])
        for qtr in range(4):
            nc.sync.dma_start(
                xcs[0][:, qtr * KK // 4:(qtr + 1) * KK // 4],
                x_d[:, 0, qtr * KK // 4:(qtr + 1) * KK // 4])
        for mi in range(1, NH):
            nc.sync.dma_start(wq_t[mi][:], wq_d[:, mi])
        for mi in range(NH):
            nc.sync.dma_start(wk_t[mi][:], wk_d[:, mi])
        nc.sync.dma_start(wv_sb[:], wv_d[:])
        nc.sync.dma_start(mask_sb[:], mask_d[:])

        for nj in range(NJ):
            xc = xcs.get(nj)
            if xc is None:
                xc = xpool.tile([P, KK, 512], bf16, name=f"{r}xc{nj}",
                                tag="xc")
                nc.sync.dma_start(xc[:], x_d[:, nj])
            # qT, kT: [CH, S] channel-major (per head = 128 partitions)
            for w_t, dst in ((wq_t, q_sb), (wk_t, k_sb)):
                for mi in range(NH):
                    acc = psA.tile([P, 512], f32, name=f"{r}qk{nj}_{mi}",
                                   tag="accA")
                    for kk in range(KK):
                        nc.tensor.matmul(
                            acc[:], w_t[mi][:, kk, :], xc[:, kk, :],
                            start=(kk == 0), stop=(kk == KK - 1))
                    nc.scalar.copy(
                        out=dst[:, mi, nj * 512:(nj + 1) * 512],
                        in_=acc[:])
            # v: [S, CH] row-major (keys on partitions)
            for si in range(4):
                sg = 4 * nj + si
                acc = psA.tile([P, CH], f32, name=f"{r}v{sg}", tag="accA")
                for kk in range(KK):
                    nc.tensor.matmul(
                        acc[:], xc[:, kk, si * P:(si + 1) * P],
                        wv_sb[:, kk, :],
                        start=(kk == 0), stop=(kk == KK - 1))
                nc.scalar.copy(out=v_sb[:, sg, :], in_=acc[:])

    # wo reuses wv's slot (Tile serializes the DMA after last wv read)
    wo_sb = pp.tile([P, NH, NJ, 512], bf16, name=r + "wo_sb", tag="wvwo")
    nc.sync.dma_start(wo_sb[:], wo_d[:])

    # ---------- Phase 2+3: attention + output projection ----------
    ADD = mybir.AluOpType.add
    eng_toggle = [0]

    with (
        tc.tile_pool(name=r + "apool", bufs=12) as apool,
        tc.tile_pool(name=r + "tpool", bufs=6) as tpool,
        tc.tile_pool(name=r + "rpool", bufs=3) as rpool,
        tc.tile_pool(name=r + "ostage", bufs=4) as ostage,
    ):
        for qc in range(NJ):          # query chunk of 512
            qs, qe = qc * 512, (qc + 1) * 512
            ktmax = 4 * qc + 4        # causal: key tiles 0..ktmax-1
            for h in range(NH):
                # scoresT [keys=128, queries=512], two key tiles per 2-bank
                # PSUM tile so exp runs as one [128,1024] ACT op
                a_slices = []
                for kt0 in range(0, ktmax, 2):
                    pw = psW.tile([P, 1024], f32,
                                  name=f"{r}st{qc}_{h}_{kt0}", tag="accW")
                    for j2 in (0, 1):
                        kt = kt0 + j2
                        nc.tensor.matmul(
                            pw[:, j2 * 512:(j2 + 1) * 512],
                            k_sb[:, h, kt * P:(kt + 1) * P],
                            q_sb[:, h, qs:qe], start=True, stop=True)
                    a2 = apool.tile([P, 1024], bf16,
                                    name=f"{r}a{qc}_{h}_{kt0}", tag="a")
                    nc.scalar.activation(a2[:], pw[:], EXP, bias=zbias[:])
                    for j2 in (0, 1):
                        kt = kt0 + j2
                        sl = a2[:, j2 * 512:(j2 + 1) * 512]
                        if kt >= 4 * qc:  # diagonal tile: causal 0/1 mask
                            nc.vector.tensor_mul(
                                out=sl, in0=sl,
                                in1=mask_sb[:, kt - 4 * qc, :])
                        a_slices.append(sl)
                # oT accumulation: [HD, 512] += v_kt^T-contract a_kt
                po = psB.tile([P, 512], f32, name=f"{r}po{qc}_{h}",
                              tag="po")
                for kt in range(ktmax):
                    nc.tensor.matmul(
                        po[:], v_sb[:, kt, h * HD:(h + 1) * HD],
                        a_slices[kt],
                        start=(kt == 0), stop=(kt == ktmax - 1))
                # normalizer: column sums of a over all key tiles, replicated
                # to all partitions by the all-ones lhsT
                pn = psB.tile([P, 512], f32, name=f"{r}pn{qc}_{h}",
                              tag="pn")
                if SUM_MODE == "pair":
                    # halve the PE sum-matmuls: pair-add on DVE/GpSimd first
                    pairs = []
                    for i in range(0, ktmax, 2):
                        t = tpool.tile([P, 512], bf16,
                                       name=f"{r}ts{qc}_{h}_{i}", tag="tsum")
                        eng = (nc.vector if eng_toggle[0] % 2 == 0
                               else nc.gpsimd)
                        eng_toggle[0] += 1
                        eng.tensor_tensor(t[:], a_slices[i],
                                          a_slices[i + 1], ADD)
                        pairs.append(t[:])
                    sum_rhs = pairs
                else:
                    sum_rhs = a_slices
                for i, t in enumerate(sum_rhs):
                    nc.tensor.matmul(pn[:], ones_sb[:], t,
                                     start=(i == 0),
                                     stop=(i == len(sum_rhs) - 1))
                rec = rpool.tile([P, 512], f32, name=f"{r}rc{qc}_{h}",
                                 tag="rec")
                nc.vector.reciprocal_approx_fast(rec[:], pn[:])
                nc.vector.tensor_mul(out=o_sb[:, h, qs:qe],
                                     in0=po[:], in1=rec[:])

            # output projection, delayed one chunk so it never waits on the
            # o values produced just above
            for pqc in ([qc - 1] if qc > 0 else []) + ([qc] if qc == NJ - 1
                                                       else []):
                _emit_proj(nc, psA, ostage, o_sb, wo_sb, out_d, pqc, r)


def _emit_proj(nc, psA, ostage, o_sb, wo_sb, out_d, qc, r):
    import concourse.mybir as mybir
    f32 = mybir.dt.float32
    bf16 = mybir.dt.bfloat16
    for si in range(4 * qc, 4 * qc + 4):
        for ncol in range(NJ):
            acc = psA.tile([P, 512], f32,
                           name=f"{r}pr{si}_{ncol}", tag="accA")
            for h in range(NH):
                nc.tensor.matmul(
                    acc[:], o_sb[:, h, si * P:(si + 1) * P],
                    wo_sb[:, h, ncol, :],
                    start=(h == 0), stop=(h == NH - 1))
            stg = ostage.tile([P, 512], bf16,
                              name=f"{r}os{si}_{ncol}", tag="os")
            nc.vector.tensor_copy(out=stg[:], in_=acc[:])
            nc.sync.dma_start(
                out_d[:, si, ncol * 512:(ncol + 1) * 512],
                stg[:])


def _pack_inputs(x, Wqkv, Wo):
    """Host-side shard + pack into the per-core DMA-friendly layouts.
    Arrays are shared between cores where identical (x per batch, weights
    per TP group, masks global)."""
    scale = np.float32(HD) ** np.float32(-0.5)
    masks = np.zeros((P, NH, 512), dtype=BF16)
    k_idx = np.arange(P)[:, None]
    q_idx = np.arange(512)[None, :]
    for j in range(NH):
        masks[:, j, :] = (P * j + k_idx <= q_idx).astype(BF16)

    xps = []
    for b in range(B):
        xb = np.asarray(x[b], dtype=np.float32)
        # xT packed: [p, nj, kk, q] with xT[128*kk+p, 512*nj+q] = xb[q', d']
        xps.append(np.ascontiguousarray(
            xb.astype(BF16).reshape(NJ, 512, KK, P).transpose(3, 0, 2, 1)))

    wmaps = []
    for g in range(G):
        wq = (np.asarray(Wqkv[:, CH * g:CH * (g + 1)], np.float32) * scale)
        wk = np.asarray(Wqkv[:, D + CH * g:D + CH * (g + 1)], np.float32)
        wv = np.asarray(Wqkv[:, 2 * D + CH * g:2 * D + CH * (g + 1)],
                        np.float32)
        wo = np.asarray(Wo[CH * g:CH * (g + 1), :], np.float32)
        wmaps.append({
            "wq": np.ascontiguousarray(
                wq.astype(BF16).reshape(KK, P, NH, P).transpose(1, 2, 0, 3)),
            "wk": np.ascontiguousarray(
                wk.astype(BF16).reshape(KK, P, NH, P).transpose(1, 2, 0, 3)),
            "wv": np.ascontiguousarray(
                wv.astype(BF16).reshape(KK, P, CH).transpose(1, 0, 2)),
            "wo": np.ascontiguousarray(
                wo.astype(BF16).reshape(NH, P, NJ, 512).transpose(1, 0, 2, 3)),
        })

    return [{"x": xps[c // G], "masks": masks, **wmaps[c % G]}
            for c in range(8)]


def _unpack_outputs(results):
    """Sum the 4 TP partials per batch and restore [B, S, D]."""
    out = np.zeros((B, S, D), dtype=np.float32)
    for c, res in enumerate(results):
        b = c // G
        part = np.asarray(res["out"]).astype(np.float32)   # [p, si, col]
        out[b] += part.transpose(1, 0, 2).reshape(S, D)
    return out


def kernel(x, Wqkv, Wo, _trace=False, _trace_kwargs=None):
    from concourse import bass_utils

    nc = _build()
    in_maps = _pack_inputs(x, Wqkv, Wo)
    res = bass_utils.run_bass_kernel_spmd(
        nc, in_maps, core_ids=list(range(8)), trace=_trace,
        **(_trace_kwargs or {}))
    out = _unpack_outputs(res.results)
    if _trace:
        kernel.last_result = res
    return out
